# revision 7
# baseline (speedup 1.0000x reference)
"""MicroGCN on 8 Trainium2 NeuronCores (Bass/Tile).

Strategy (v5):
  - Nodes dst-sharded 8 ways (12500/core). Edges (incl. self-loops) assigned
    to the core owning their dst.
  - Per core, nodes are sorted by in-degree (desc) and grouped into NGRP=25
    groups of 512 nodes; rank r -> group r//512, slot r%512. Group g gets
    T_g = max in-degree in group (max across cores so all cores share one
    schedule); each node's edges sit in its own slot column across tiles
    t=0..deg-1, empty slots zero. Degree sorting keeps padding to a few %.
  - Host pre-computes msg rows x[src]*norm*16 (f32 mult) stored as fp8-e4m3
    (the x16 prescale keeps values in e4m3 normal range; exactly compensated
    by the ACT relu scale=1/16). Layout is feature-major G[f, (tile, slot)]
    so each tile is a contiguous [128f, 512slot] moving operand, streamed
    with ~3MB HWDGE DMAs.
  - Layer 1 collapses gather+scatter+linear into one accumulation:
    u1T[u, 512d] += W1^T @ g_tile (W1 bf16 stationary, fp8 moving operand,
    PSUM fp32 accumulates over the group's T_g tiles = the scatter-add and
    the layer-1 matmul in one). ACT relu(u1T/16 + b1) -> h1'T bf16.
    Per 128-wide sub-block: h2 = h1'T^T @ W2 -> PSUM -> SBUF bf16.
  - Layer 2: no gathers. Host builds dense P[slot, s] = sum of norm over
    edges with src=node(slot) grouped by state[dst] (src-sharded; same cores
    own the same nodes). T2[64,64] += P_b^T @ h2_b accumulated in PSUM.
  - Host: degree/norm precompute, packing, final sum over cores / counts + b2.
"""
import sys

sys.path.insert(0, "/opt/trn_rl_repo")

import numpy as np
import ml_dtypes

import concourse.bacc as bacc
import concourse.mybir as mybir
import concourse.tile as tile
from concourse.bass_utils import run_bass_kernel_spmd

F32 = mybir.dt.float32
BF16 = mybir.dt.bfloat16
FP8 = mybir.dt.float8e4
BF = ml_dtypes.bfloat16
F8 = ml_dtypes.float8_e4m3
MSG_SCALE = 16.0   # prescale into e4m3 normal range; compensated in ACT scale

N = 100_000
E = 1_600_000
S = 64
IN_DIM = 128
HID_DIM = 128
OUT_DIM = 64
NCORES = 8
NPC = N // NCORES              # nodes per core
P128 = 128
GW = 512                       # node-group width (one PSUM bank of fp32)
NSUB = GW // P128              # 128-wide sub-blocks per group
NGRP = (NPC + GW - 1) // GW    # 25 groups per core
NBLK = NGRP * NSUB             # 100 sub-blocks (layer-2 granularity)
CTMAX = 44                     # max [128,512] tiles per G-stream chunk (~2.9MB)

_compiled = None  # (nc, schedule)


def _prepare(x, edge_src, edge_dst, edge_weight, state, W1, b1, W2, b2):
    x = np.asarray(x, np.float32)
    src = np.asarray(edge_src, np.int64)
    dst = np.asarray(edge_dst, np.int64)
    w = np.asarray(edge_weight, np.float32)
    state = np.asarray(state, np.int64)

    loop = np.arange(N, dtype=np.int64)
    src2 = np.concatenate([src, loop])
    dst2 = np.concatenate([dst, loop])
    w2 = np.concatenate([w, np.ones(N, np.float32)])

    deg = np.bincount(dst2, weights=w2, minlength=N).astype(np.float32)
    dinv = np.where(deg > 0, 1.0 / np.sqrt(deg), 0.0).astype(np.float32)
    norm = (dinv[src2] * w2 * dinv[dst2]).astype(np.float32)

    indeg = np.bincount(dst2, minlength=N).astype(np.int64)  # includes self
    indeg_l = indeg.reshape(NCORES, NPC)

    # ---- degree-sorted packing: rank r -> group r//GW, slot r%GW ----
    rank2node = np.argsort(-indeg_l, axis=1, kind="stable")  # [c, r] -> local
    grp_of = np.empty((NCORES, NPC), np.int64)
    slot_of = np.empty((NCORES, NPC), np.int64)
    r = np.arange(NPC)
    Tper = np.zeros((NCORES, NGRP), np.int64)
    for c in range(NCORES):
        grp_of[c, rank2node[c]] = r // GW
        slot_of[c, rank2node[c]] = r % GW
        sd = indeg_l[c, rank2node[c]]           # descending degrees
        for g in range(NGRP):
            gd = sd[g * GW:(g + 1) * GW]
            Tper[c, g] = gd[0] if len(gd) else 1
    Tsched = np.maximum(Tper.max(axis=0), 1)     # shared schedule
    assert Tsched.max() <= CTMAX
    tstart = np.concatenate([[0], np.cumsum(Tsched)]).astype(np.int64)
    NTILES = int(tstart[-1])

    # ---- within-dst edge index t (0..indeg-1) ----
    od = np.argsort(dst2, kind="stable")
    dst_sorted = dst2[od]
    dcnt = np.bincount(dst2, minlength=N)
    dstarts = np.concatenate([[0], np.cumsum(dcnt)[:-1]])
    tcnt = np.arange(len(od)) - dstarts[dst_sorted]

    # ---- per-core G slab: G[c][f, (tstart[g]+t)*GW + slot] ----
    G = np.empty((NCORES, P128, NTILES * GW), dtype=F8)
    for c in range(NCORES):
        lo = np.searchsorted(dst_sorted, c * NPC)
        hi = np.searchsorted(dst_sorted, (c + 1) * NPC)
        e_idx = od[lo:hi]
        dl = dst_sorted[lo:hi] - c * NPC
        t = tcnt[lo:hi]
        gg = grp_of[c, dl]
        sl = slot_of[c, dl]
        assert (t < Tsched[gg]).all()
        col = (tstart[gg] + t) * GW + sl
        msg = (x[src2[e_idx]] * (MSG_SCALE * norm[e_idx])[:, None]).astype(F8)
        G2f = np.zeros((NTILES * GW, P128), dtype=F8)
        G2f[col] = msg
        G[c] = np.ascontiguousarray(G2f.T)

    # ---- layer-2 P slab: P2[c][slot128, blk*S + s], blk = g*NSUB + slot//128
    core_of = np.repeat(np.arange(NCORES), NPC)
    c_src = core_of[src2]
    local = src2 - c_src * NPC
    g_s = grp_of[c_src, local]
    sl_s = slot_of[c_src, local]
    blk = g_s * NSUB + sl_s // P128
    srow = c_src * (NBLK * P128) + blk * P128 + (sl_s % P128)
    flat = srow * S + state[dst2]
    Pm = np.bincount(flat, weights=norm,
                     minlength=NCORES * NBLK * P128 * S)
    Pm = Pm.reshape(NCORES, NBLK, P128, S)
    P2 = np.ascontiguousarray(
        Pm.transpose(0, 2, 1, 3).reshape(NCORES, P128, NBLK * S)).astype(BF)

    counts = np.bincount(state, minlength=S).astype(np.float32)

    # greedy chunking of groups into <=CTMAX-tile G-stream chunks,
    # with small first chunks so the PE starts sooner
    chunks = []
    cur, cur_t = [], 0
    ramp = [CTMAX // 4, CTMAX // 2]
    for g in range(NGRP):
        budget = ramp[len(chunks)] if len(chunks) < len(ramp) else CTMAX
        if cur and cur_t + Tsched[g] > budget:
            chunks.append(cur)
            cur, cur_t = [], 0
        cur.append(g)
        cur_t += int(Tsched[g])
    chunks.append(cur)

    return dict(
        Tsched=tuple(int(v) for v in Tsched), chunks=chunks,
        G=G, P2=P2, counts=counts,
        W1=np.asarray(W1, np.float32).astype(BF),
        b1=np.asarray(b1, np.float32).reshape(P128, 1),
        W2=np.asarray(W2, np.float32).astype(BF),
        b2=np.asarray(b2, np.float32),
    )


def _build(Tsched, chunks):
    nc = bacc.Bacc("TRN2")
    tstart = np.concatenate([[0], np.cumsum(Tsched)]).astype(np.int64)
    NTILES = int(tstart[-1])
    G_d = nc.dram_tensor("G", [P128, NTILES * GW], FP8, kind="ExternalInput")
    P2_d = nc.dram_tensor("P2", [P128, NBLK * S], BF16, kind="ExternalInput")
    W1_d = nc.dram_tensor("W1", [IN_DIM, HID_DIM], BF16, kind="ExternalInput")
    b1_d = nc.dram_tensor("b1", [P128, 1], F32, kind="ExternalInput")
    W2_d = nc.dram_tensor("W2", [HID_DIM, OUT_DIM], BF16, kind="ExternalInput")
    T2_d = nc.dram_tensor("T2", [S, OUT_DIM], F32, kind="ExternalOutput")

    with tile.TileContext(nc) as tc:
        with (
            tc.tile_pool(name="const", bufs=1) as constp,
            tc.tile_pool(name="gch", bufs=3) as gp,
            tc.tile_pool(name="blk", bufs=3) as blkp,
            tc.tile_pool(name="ps", bufs=2, space="PSUM") as psp,
            tc.tile_pool(name="ps2", bufs=2, space="PSUM") as ps2p,
            tc.tile_pool(name="psT2", bufs=1, space="PSUM") as psT2p,
        ):
            P2_sb = constp.tile([P128, NBLK * S], BF16, tag="P2")
            W1_sb = constp.tile([IN_DIM, HID_DIM], BF16, tag="W1")
            b1_sb = constp.tile([P128, 1], F32, tag="b1")
            W2_sb = constp.tile([HID_DIM, OUT_DIM], BF16, tag="W2")
            nc.scalar.dma_start(out=W1_sb[:], in_=W1_d[:])
            nc.scalar.dma_start(out=b1_sb[:], in_=b1_d[:])
            nc.scalar.dma_start(out=W2_sb[:], in_=W2_d[:])
            nc.scalar.dma_start(out=P2_sb[:], in_=P2_d[:])

            T2_ps = psT2p.tile([S, OUT_DIM], F32, tag="T2", space="PSUM")
            for ch in chunks:
                ct = sum(Tsched[g] for g in ch)
                c0 = int(tstart[ch[0]])
                gch = gp.tile([P128, CTMAX * GW], FP8, tag="gch")
                nc.sync.dma_start(
                    out=gch[:, :ct * GW],
                    in_=G_d[:, c0 * GW:(c0 + ct) * GW])
                off = 0
                for g in ch:
                    Tg = Tsched[g]
                    u1T_ps = psp.tile([P128, GW], F32, tag="u1T",
                                      space="PSUM")
                    for t in range(Tg):
                        nc.tensor.matmul(
                            out=u1T_ps[:], lhsT=W1_sb[:],
                            rhs=gch[:, (off + t) * GW:(off + t + 1) * GW],
                            start=(t == 0), stop=(t == Tg - 1))
                    off += Tg
                    h1pT_sb = blkp.tile([P128, GW], BF16, tag="h1pT")
                    nc.scalar.activation(
                        out=h1pT_sb[:], in_=u1T_ps[:],
                        func=mybir.ActivationFunctionType.Relu,
                        bias=b1_sb[:, 0:1], scale=1.0 / MSG_SCALE)
                    for sub in range(NSUB):
                        b = g * NSUB + sub
                        h2_ps = ps2p.tile([P128, OUT_DIM], F32, tag="h2ps",
                                          space="PSUM")
                        nc.tensor.matmul(
                            out=h2_ps[:],
                            lhsT=h1pT_sb[:, sub * P128:(sub + 1) * P128],
                            rhs=W2_sb[:], start=True, stop=True)
                        h2blk = blkp.tile([P128, OUT_DIM], BF16, tag="h2blk")
                        nc.vector.tensor_copy(out=h2blk[:], in_=h2_ps[:])
                        nc.tensor.matmul(
                            out=T2_ps[:], lhsT=P2_sb[:, b * S:(b + 1) * S],
                            rhs=h2blk[:],
                            start=(b == 0), stop=(b == NBLK - 1))
            T2_sb = blkp.tile([S, OUT_DIM], F32, tag="T2sb")
            nc.vector.tensor_copy(out=T2_sb[:], in_=T2_ps[:])
            nc.sync.dma_start(out=T2_d[:], in_=T2_sb[:])

    nc.compile()
    return nc


def kernel(x, edge_src, edge_dst, edge_weight, state, W1, b1, W2, b2,
           trace=False):
    global _compiled
    prep = _prepare(x, edge_src, edge_dst, edge_weight, state, W1, b1, W2, b2)
    key = prep["Tsched"]
    if _compiled is None or _compiled[1] != key:
        _compiled = (_build(prep["Tsched"], prep["chunks"]), key)
    nc = _compiled[0]

    in_maps = []
    for c in range(NCORES):
        in_maps.append({
            "G": prep["G"][c],
            "P2": prep["P2"][c],
            "W1": prep["W1"],
            "b1": prep["b1"],
            "W2": prep["W2"],
        })
    res = run_bass_kernel_spmd(nc, in_maps, core_ids=list(range(NCORES)),
                               trace=trace)
    T2 = np.zeros((S, OUT_DIM), np.float64)
    for c in range(NCORES):
        T2 += res.results[c]["T2"].astype(np.float64)
    counts = prep["counts"].astype(np.float64)
    out = T2 / np.maximum(counts, 1.0)[:, None]
    out = out + (counts > 0)[:, None] * prep["b2"].astype(np.float64)
    out = out.astype(np.float32)
    if trace:
        return out, res
    return out


# revision 8
# speedup vs baseline: 1.0078x; 1.0078x over previous
"""MicroGCN on 8 Trainium2 NeuronCores (Bass/Tile).

Strategy (v6):
  - Nodes dst-sharded 8 ways (12500/core). Edges (incl. self-loops) assigned
    to the core owning their dst.
  - Per core, nodes are sorted by in-degree (ascending) into NBLK=98 blocks
    of 128 nodes; rank r -> block r//128, partition slot r%128. Block b gets
    T_b = max in-degree in block (max across cores so all cores share one
    schedule); each node's edges sit in its own slot column across tiles
    t=0..deg-1, empty slots zero. Degree sorting keeps padding to a few %.
  - Host pre-computes msg rows x[src]*norm*16 (f32 mult) stored as fp8-e4m3
    (the x16 prescale keeps values in e4m3 normal range; exactly compensated
    by the ACT relu scale=1/16). Layout is feature-major G[f, (tile, slot)]
    so each tile is a contiguous [128f, 128slot] stationary operand,
    streamed with ~2.6MB HWDGE DMAs.
  - Layer 1 collapses gather+scatter+linear into one accumulation:
    u1[d, u] += g_tile^T @ W1 (g fp8 stationary, W1 bf16 moving, PSUM fp32
    accumulates over the block's T_b tiles = the scatter-add and the layer-1
    matmul in one). b1 enters as a K=1 rank-1 matmul (ones^T @ (16*b1)) in
    the same accumulation group. ACT relu(u1/16) -> h1p[d, u] bf16.
  - Layer 2 + pooling are reassociated: T2 = P^T @ (h1p @ W2)
    = (sum_b P_b^T @ h1p_b) @ W2, so per block only one matmul
    QT[u, s] += h1p_b^T @ P_b accumulates in a single PSUM bank, and W2 is
    applied once at the end: T2 = QT^T @ W2. Host builds dense
    P[slot, s] = sum of norm over edges with src=node(slot), state[dst]=s.
  - Host: degree/norm precompute, packing, final sum over cores / counts + b2.
"""
import sys

sys.path.insert(0, "/opt/trn_rl_repo")

import numpy as np
import ml_dtypes

import concourse.bacc as bacc
import concourse.mybir as mybir
import concourse.tile as tile
from concourse.bass_utils import run_bass_kernel_spmd

F32 = mybir.dt.float32
BF16 = mybir.dt.bfloat16
FP8 = mybir.dt.float8e4
BF = ml_dtypes.bfloat16
F8 = ml_dtypes.float8_e4m3
MSG_SCALE = 16.0   # prescale into e4m3 normal range; compensated in ACT scale

N = 100_000
E = 1_600_000
S = 64
IN_DIM = 128
HID_DIM = 128
OUT_DIM = 64
NCORES = 8
NPC = N // NCORES              # nodes per core
P128 = 128
NBLK = (NPC + P128 - 1) // P128  # 98 blocks per core
CTMAX = 160                    # max [128,128] tiles per G-stream chunk (~2.6MB)

_compiled = None  # (nc, schedule)


def _prepare(x, edge_src, edge_dst, edge_weight, state, W1, b1, W2, b2):
    x = np.asarray(x, np.float32)
    src = np.asarray(edge_src, np.int64)
    dst = np.asarray(edge_dst, np.int64)
    w = np.asarray(edge_weight, np.float32)
    state = np.asarray(state, np.int64)

    loop = np.arange(N, dtype=np.int64)
    src2 = np.concatenate([src, loop])
    dst2 = np.concatenate([dst, loop])
    w2 = np.concatenate([w, np.ones(N, np.float32)])

    deg = np.bincount(dst2, weights=w2, minlength=N).astype(np.float32)
    dinv = np.where(deg > 0, 1.0 / np.sqrt(deg), 0.0).astype(np.float32)
    norm = (dinv[src2] * w2 * dinv[dst2]).astype(np.float32)

    indeg = np.bincount(dst2, minlength=N).astype(np.int64)  # includes self
    indeg_l = indeg.reshape(NCORES, NPC)

    # ---- degree-sorted packing (ascending): rank r -> block r//128 ----
    rank2node = np.argsort(indeg_l, axis=1, kind="stable")  # [c, r] -> local
    bin_of = np.empty((NCORES, NPC), np.int64)
    slot_of = np.empty((NCORES, NPC), np.int64)
    r = np.arange(NPC)
    Tper = np.zeros((NCORES, NBLK), np.int64)
    for c in range(NCORES):
        bin_of[c, rank2node[c]] = r // P128
        slot_of[c, rank2node[c]] = r % P128
        sd = indeg_l[c, rank2node[c]]           # ascending degrees
        for b in range(NBLK):
            bd = sd[b * P128:(b + 1) * P128]
            Tper[c, b] = bd[-1] if len(bd) else 1
    Tsched = np.maximum(Tper.max(axis=0), 1)     # shared schedule
    assert Tsched.max() <= CTMAX
    tstart = np.concatenate([[0], np.cumsum(Tsched)]).astype(np.int64)
    NTILES = int(tstart[-1])

    # ---- within-dst edge index t (0..indeg-1) ----
    od = np.argsort(dst2, kind="stable")
    dst_sorted = dst2[od]
    dcnt = np.bincount(dst2, minlength=N)
    dstarts = np.concatenate([[0], np.cumsum(dcnt)[:-1]])
    tcnt = np.arange(len(od)) - dstarts[dst_sorted]

    # ---- per-core G slab: G[c][f, (tstart[b]+t)*128 + slot] ----
    G = np.empty((NCORES, P128, NTILES * P128), dtype=F8)
    for c in range(NCORES):
        lo = np.searchsorted(dst_sorted, c * NPC)
        hi = np.searchsorted(dst_sorted, (c + 1) * NPC)
        e_idx = od[lo:hi]
        dl = dst_sorted[lo:hi] - c * NPC
        t = tcnt[lo:hi]
        bb = bin_of[c, dl]
        sl = slot_of[c, dl]
        assert (t < Tsched[bb]).all()
        col = (tstart[bb] + t) * P128 + sl
        msg = (x[src2[e_idx]] * (MSG_SCALE * norm[e_idx])[:, None]).astype(F8)
        G2f = np.zeros((NTILES * P128, P128), dtype=F8)
        G2f[col] = msg
        G[c] = np.ascontiguousarray(G2f.T)

    # ---- layer-2 P slab: P2[c][slot, b*S + s] ----
    core_of = np.repeat(np.arange(NCORES), NPC)
    c_src = core_of[src2]
    local = src2 - c_src * NPC
    b_s = bin_of[c_src, local]
    sl_s = slot_of[c_src, local]
    srow = c_src * (NBLK * P128) + b_s * P128 + sl_s
    flat = srow * S + state[dst2]
    Pm = np.bincount(flat, weights=norm,
                     minlength=NCORES * NBLK * P128 * S)
    Pm = Pm.reshape(NCORES, NBLK, P128, S)
    P2 = np.ascontiguousarray(
        Pm.transpose(0, 2, 1, 3).reshape(NCORES, P128, NBLK * S)).astype(BF)

    counts = np.bincount(state, minlength=S).astype(np.float32)

    # greedy chunking of blocks into <=CTMAX-tile G-stream chunks; the first
    # chunks are small (low-degree blocks first) so the PE starts early
    chunks = []
    cur, cur_t = [], 0
    ramp = [CTMAX // 8, CTMAX // 3]
    for b in range(NBLK):
        budget = ramp[len(chunks)] if len(chunks) < len(ramp) else CTMAX
        if cur and cur_t + Tsched[b] > budget:
            chunks.append(cur)
            cur, cur_t = [], 0
        cur.append(b)
        cur_t += int(Tsched[b])
    chunks.append(cur)

    return dict(
        Tsched=tuple(int(v) for v in Tsched), chunks=chunks,
        G=G, P2=P2, counts=counts,
        W1=np.asarray(W1, np.float32).astype(BF),
        b1x16=(np.asarray(b1, np.float32) * MSG_SCALE
               ).astype(BF).reshape(1, HID_DIM),
        ones=np.ones((1, P128), dtype=BF),
        W2=np.asarray(W2, np.float32).astype(BF),
        b2=np.asarray(b2, np.float32),
    )


def _build(Tsched, chunks):
    nc = bacc.Bacc("TRN2")
    tstart = np.concatenate([[0], np.cumsum(Tsched)]).astype(np.int64)
    NTILES = int(tstart[-1])
    G_d = nc.dram_tensor("G", [P128, NTILES * P128], FP8, kind="ExternalInput")
    P2_d = nc.dram_tensor("P2", [P128, NBLK * S], BF16, kind="ExternalInput")
    W1_d = nc.dram_tensor("W1", [IN_DIM, HID_DIM], BF16, kind="ExternalInput")
    b1x16_d = nc.dram_tensor("b1x16", [1, HID_DIM], BF16, kind="ExternalInput")
    ones_d = nc.dram_tensor("ones", [1, P128], BF16, kind="ExternalInput")
    W2_d = nc.dram_tensor("W2", [HID_DIM, OUT_DIM], BF16, kind="ExternalInput")
    T2_d = nc.dram_tensor("T2", [S, OUT_DIM], F32, kind="ExternalOutput")

    with tile.TileContext(nc) as tc:
        with (
            tc.tile_pool(name="const", bufs=1) as constp,
            tc.tile_pool(name="gch", bufs=3) as gp,
            tc.tile_pool(name="blk", bufs=3) as blkp,
            tc.tile_pool(name="ps", bufs=2, space="PSUM") as psp,
            tc.tile_pool(name="psQ", bufs=1, space="PSUM") as psQp,
            tc.tile_pool(name="psT2", bufs=1, space="PSUM") as psT2p,
        ):
            P2_sb = constp.tile([P128, NBLK * S], BF16, tag="P2")
            W1_sb = constp.tile([IN_DIM, HID_DIM], BF16, tag="W1")
            b1x16_sb = constp.tile([1, HID_DIM], BF16, tag="b1x16")
            ones_sb = constp.tile([1, P128], BF16, tag="ones")
            W2_sb = constp.tile([HID_DIM, OUT_DIM], BF16, tag="W2")
            # small consts first on the sync queue (ahead of G chunk 0);
            # P2 follows chunk 0 (first needed after block 0's relu)
            nc.sync.dma_start(out=W1_sb[:], in_=W1_d[:])
            nc.sync.dma_start(out=b1x16_sb[:], in_=b1x16_d[:])
            nc.sync.dma_start(out=ones_sb[:], in_=ones_d[:])
            nc.sync.dma_start(out=W2_sb[:], in_=W2_d[:])

            QT_ps = psQp.tile([HID_DIM, S], F32, tag="QT", space="PSUM")
            first = True
            for ch in chunks:
                ct = sum(Tsched[b] for b in ch)
                c0 = int(tstart[ch[0]])
                gch = gp.tile([P128, CTMAX * P128], FP8, tag="gch")
                nc.sync.dma_start(
                    out=gch[:, :ct * P128],
                    in_=G_d[:, c0 * P128:(c0 + ct) * P128])
                if first:
                    nc.sync.dma_start(out=P2_sb[:], in_=P2_d[:])
                    first = False
                off = 0
                for b in ch:
                    Tb = Tsched[b]
                    u1_ps = psp.tile([P128, HID_DIM], F32, tag="u1",
                                     space="PSUM")
                    # b1 as a K=1 rank-1 update opens the accumulation group
                    nc.tensor.matmul(out=u1_ps[:], lhsT=ones_sb[:],
                                     rhs=b1x16_sb[:], start=True, stop=False)
                    for t in range(Tb):
                        nc.tensor.matmul(
                            out=u1_ps[:],
                            lhsT=gch[:, (off + t) * P128:(off + t + 1) * P128],
                            rhs=W1_sb[:],
                            start=False, stop=(t == Tb - 1))
                    off += Tb
                    h1p_sb = blkp.tile([P128, HID_DIM], BF16, tag="h1p")
                    nc.scalar.activation(
                        out=h1p_sb[:], in_=u1_ps[:],
                        func=mybir.ActivationFunctionType.Relu,
                        bias=0.0, scale=1.0 / MSG_SCALE)
                    nc.tensor.matmul(
                        out=QT_ps[:], lhsT=h1p_sb[:],
                        rhs=P2_sb[:, b * S:(b + 1) * S],
                        start=(b == 0), stop=(b == NBLK - 1))
            QT_sb = blkp.tile([HID_DIM, S], BF16, tag="QTsb")
            nc.vector.tensor_copy(out=QT_sb[:], in_=QT_ps[:])
            T2_ps = psT2p.tile([S, OUT_DIM], F32, tag="T2", space="PSUM")
            nc.tensor.matmul(out=T2_ps[:], lhsT=QT_sb[:], rhs=W2_sb[:],
                             start=True, stop=True)
            T2_sb = blkp.tile([S, OUT_DIM], F32, tag="T2sb")
            nc.vector.tensor_copy(out=T2_sb[:], in_=T2_ps[:])
            nc.sync.dma_start(out=T2_d[:], in_=T2_sb[:])

    nc.compile()
    return nc


def kernel(x, edge_src, edge_dst, edge_weight, state, W1, b1, W2, b2,
           trace=False):
    global _compiled
    prep = _prepare(x, edge_src, edge_dst, edge_weight, state, W1, b1, W2, b2)
    key = prep["Tsched"]
    if _compiled is None or _compiled[1] != key:
        _compiled = (_build(prep["Tsched"], prep["chunks"]), key)
    nc = _compiled[0]

    in_maps = []
    for c in range(NCORES):
        in_maps.append({
            "G": prep["G"][c],
            "P2": prep["P2"][c],
            "W1": prep["W1"],
            "b1x16": prep["b1x16"],
            "ones": prep["ones"],
            "W2": prep["W2"],
        })
    res = run_bass_kernel_spmd(nc, in_maps, core_ids=list(range(NCORES)),
                               trace=trace)
    T2 = np.zeros((S, OUT_DIM), np.float64)
    for c in range(NCORES):
        T2 += res.results[c]["T2"].astype(np.float64)
    counts = prep["counts"].astype(np.float64)
    out = T2 / np.maximum(counts, 1.0)[:, None]
    out = out + (counts > 0)[:, None] * prep["b2"].astype(np.float64)
    out = out.astype(np.float32)
    if trace:
        return out, res
    return out
